# revision 7
# baseline (speedup 1.0000x reference)
"""Expert-parallel MoE FFN kernel for Trainium2 (8 NeuronCores).

Strategy: each of the 8 experts lives on its own core. Rows are routed
host-side (argsort by note_type_pos), padded to a uniform capacity C,
and shipped feature-major (transposed) so the device kernel is a pure
dense 2-layer MLP with the feature dimension on SBUF partitions:

    hT = relu(W1.T @ xT + b1)     [F, C]
    yT = W2.T @ hT + b2           [H, C]

Matmuls run in float32r (tf32-like: full fp32 storage, reduced-precision
multiply at full PE rate) with fp32 PSUM accumulation. Weights are
streamed through SBUF once (F blocked into 8 blocks of 512); xT and the
y accumulator stay resident. No collectives needed.
"""

import sys

sys.path.insert(0, "/opt/trn_rl_repo")

import numpy as np

import concourse.bass as bass
import concourse.mybir as mybir
from concourse import bacc
from concourse.tile import TileContext

H = 1024
F = 4096
N_EXPERTS = 8
P = 128
KH = H // P   # 8
KF = F // P   # 32
FB = 512      # F block size (weights streamed per block)
NFB = F // FB  # 8
FC = FB // P  # 4


def _row_tiles(C):
    """Split C columns into equal chunks <=512 (multiples of 16)."""
    n = -(-C // 512)
    rw = -(-C // n)
    rw = ((rw + 15) // 16) * 16
    tiles = []
    s = 0
    while s < C:
        w = min(rw, C - s)
        tiles.append((s, w))
        s += w
    return tiles


def build_expert_kernel(C, reps=1, dt_mm=None):
    """One expert's 2-layer MLP: xT [H, C] -> yT [H, C]."""
    f32 = mybir.dt.float32
    f32r = dt_mm if dt_mm is not None else mybir.dt.float32r
    nc = bacc.Bacc(None, target_bir_lowering=False)
    xT = nc.dram_tensor("xT", [H, C], f32r, kind="ExternalInput")
    w1 = nc.dram_tensor("w1", [H, F], f32r, kind="ExternalInput")
    b1v = nc.dram_tensor("b1v", [P, KF], f32, kind="ExternalInput")
    w2 = nc.dram_tensor("w2", [F, H], f32r, kind="ExternalInput")
    b2v = nc.dram_tensor("b2v", [P, KH], f32, kind="ExternalInput")
    yT = nc.dram_tensor("yT", [H, C], f32, kind="ExternalOutput")

    tiles = _row_tiles(C)

    with TileContext(nc) as tc:
        with (
            tc.tile_pool(name="consts", bufs=1) as consts,
            tc.tile_pool(name="xp", bufs=1) as xp,
            tc.tile_pool(name="yaccp", bufs=1) as yaccp,
            tc.tile_pool(name="w1p", bufs=2) as w1p,
            tc.tile_pool(name="w2p", bufs=2) as w2p,
            tc.tile_pool(name="hp", bufs=2) as hp,
            tc.tile_pool(name="psh", bufs=2, space="PSUM") as psh,
            tc.tile_pool(name="psy", bufs=2, space="PSUM") as psy,
        ):
            b1_sb = consts.tile([P, KF], f32, tag="b1")
            nc.sync.dma_start(b1_sb[:], b1v[:, :])
            b2_sb = consts.tile([P, KH], f32, tag="b2")
            nc.sync.dma_start(b2_sb[:], b2v[:, :])

            xT_sb = xp.tile([P, KH, C], f32r, tag="xT")
            for k in range(KH):
                nc.sync.dma_start(xT_sb[:, k, :], xT[k * P:(k + 1) * P, :])

            yacc = yaccp.tile([P, KH, C], f32, tag="yacc")

            def body():
                for fb in range(NFB):
                    w1blk = w1p.tile([P, KH, FB], f32r, tag="w1blk")
                    for k in range(KH):
                        nc.sync.dma_start(
                            w1blk[:, k, :],
                            w1[k * P:(k + 1) * P, fb * FB:(fb + 1) * FB])
                    w2blk = w2p.tile([P, FC, H], f32r, tag="w2blk")
                    for fc in range(FC):
                        nc.sync.dma_start(
                            w2blk[:, fc, :],
                            w2[fb * FB + fc * P:fb * FB + (fc + 1) * P, :])
                    for (r0, rw) in tiles:
                        h_sb = hp.tile([P, FC, max(t[1] for t in tiles)],
                                       f32r, tag="h")
                        # layer 1: hT[fc] = relu(W1blk.T @ xT + b1)
                        for fc in range(FC):
                            ph = psh.tile([P, rw], f32, tag="ph")
                            for k in range(KH):
                                nc.tensor.matmul(
                                    ph[:],
                                    w1blk[:, k, fc * P:(fc + 1) * P],
                                    xT_sb[:, k, r0:r0 + rw],
                                    start=(k == 0), stop=(k == KH - 1))
                            nc.scalar.activation(
                                h_sb[:, fc, :rw], ph[:],
                                mybir.ActivationFunctionType.Relu,
                                bias=b1_sb[:, fb * FC + fc:fb * FC + fc + 1])
                        # layer 2 partial: yacc[m] (+)= W2blk.T @ hT
                        for m in range(KH):
                            py = psy.tile([P, rw], f32, tag="py")
                            for fc in range(FC):
                                nc.tensor.matmul(
                                    py[:],
                                    w2blk[:, fc, m * P:(m + 1) * P],
                                    h_sb[:, fc, :rw],
                                    start=(fc == 0), stop=(fc == FC - 1))
                            if fb == 0:
                                # fold the layer-2 bias into the first partial
                                nc.scalar.activation(
                                    yacc[:, m, r0:r0 + rw], py[:],
                                    mybir.ActivationFunctionType.Identity,
                                    bias=b2_sb[:, m:m + 1])
                            else:
                                nc.vector.tensor_add(
                                    out=yacc[:, m, r0:r0 + rw],
                                    in0=yacc[:, m, r0:r0 + rw], in1=py[:])

            for _ in range(reps):
                body()
            # writeback (pure DMA; bias already folded in)
            for m in range(KH):
                nc.sync.dma_start(yT[m * P:(m + 1) * P, :], yacc[:, m, :])
    nc.finalize()
    return nc


def _prepare(x, note_type_pos, W1, b1, W2, b2):
    """Host-side routing: sort rows by expert, pad to capacity C."""
    T = x.shape[0]
    ntp = np.asarray(note_type_pos).astype(np.int64)
    x = np.ascontiguousarray(np.asarray(x, dtype=np.float32))
    counts = np.bincount(ntp, minlength=N_EXPERTS)
    C = int(counts.max())
    n = -(-C // 512)
    rw = ((-(-C // n) + 15) // 16) * 16
    C = rw * n

    order = np.argsort(ntp, kind="stable")
    in_maps = []
    row_idx = []
    off = 0
    for e in range(N_EXPERTS):
        rows = order[off:off + counts[e]]
        off += counts[e]
        row_idx.append(rows)
        xe = np.zeros((C, H), dtype=np.float32)
        xe[:len(rows)] = x[rows]
        in_maps.append({
            "xT": np.ascontiguousarray(xe.T),
            "w1": np.ascontiguousarray(np.asarray(W1[e], dtype=np.float32)),
            "b1v": np.ascontiguousarray(
                np.asarray(b1[e], dtype=np.float32).reshape(KF, P).T),
            "w2": np.ascontiguousarray(np.asarray(W2[e], dtype=np.float32)),
            "b2v": np.ascontiguousarray(
                np.asarray(b2[e], dtype=np.float32).reshape(KH, P).T),
        })
    return in_maps, row_idx, C


def kernel(x, note_type_pos, W1, b1, W2, b2):
    in_maps, row_idx, C = _prepare(x, note_type_pos, W1, b1, W2, b2)
    nc = build_expert_kernel(C)
    from concourse.bass_utils import run_bass_kernel_spmd
    res = run_bass_kernel_spmd(nc, in_maps, core_ids=list(range(N_EXPERTS)))
    T = np.asarray(x).shape[0]
    out = np.zeros((T, H), dtype=np.float32)
    for e in range(N_EXPERTS):
        rows = row_idx[e]
        if len(rows):
            out[rows] = res.results[e]["yT"].T[:len(rows)]
    return out


# revision 13
# speedup vs baseline: 1.8534x; 1.8534x over previous
"""Expert-parallel MoE FFN kernel for Trainium2 (8 NeuronCores).

Strategy: each of the 8 experts lives on its own core. Rows are routed
host-side (argsort by note_type_pos), padded to a uniform capacity C,
and shipped feature-major (transposed) so the device kernel is a pure
dense 2-layer MLP with the feature dimension on SBUF partitions:

    hT = relu(W1.T @ xT + b1)     [F, C]
    yT = W2.T @ hT + b2           [H, C]

Matmuls run in float32r (tf32-like: full fp32 storage, reduced-precision
multiply at full PE rate) with fp32 PSUM accumulation. Weights are
streamed through SBUF once (F blocked into 8 blocks of 512); xT and the
y accumulator stay resident. No collectives needed.
"""

import sys

sys.path.insert(0, "/opt/trn_rl_repo")

import numpy as np

import concourse.bass as bass
import concourse.mybir as mybir
from concourse import bacc
from concourse.tile import TileContext

H = 1024
F = 4096
N_EXPERTS = 8
P = 128
KH = H // P   # 8
KF = F // P   # 32
FB = 512      # F block size (weights streamed per block)
NFB = F // FB  # 8
FC = FB // P  # 4


def _row_tiles(C):
    """Split C columns into equal chunks <=512 (multiples of 16)."""
    n = -(-C // 512)
    rw = -(-C // n)
    rw = ((rw + 15) // 16) * 16
    tiles = []
    s = 0
    while s < C:
        w = min(rw, C - s)
        tiles.append((s, w))
        s += w
    return tiles


def build_expert_kernel(C, reps=1, dt_mm=None):
    """One expert's 2-layer MLP: xT [H, C] -> yT [H, C]."""
    f32 = mybir.dt.float32
    f32r = dt_mm if dt_mm is not None else mybir.dt.float32r
    nc = bacc.Bacc(None, target_bir_lowering=False)
    xT = nc.dram_tensor("xT", [H, C], f32r, kind="ExternalInput")
    w1 = nc.dram_tensor("w1", [H, F], f32r, kind="ExternalInput")
    b1v = nc.dram_tensor("b1v", [P, KF], f32, kind="ExternalInput")
    w2 = nc.dram_tensor("w2", [F, H], f32r, kind="ExternalInput")
    b2v = nc.dram_tensor("b2v", [P, KH], f32, kind="ExternalInput")
    yT = nc.dram_tensor("yT", [H, C], f32, kind="ExternalOutput")

    tiles = _row_tiles(C)
    # xT+yacc residency is 64*C B/partition; drop prefetch depth when a
    # pathological routing pushes C past what bufs=3 pools leave room for.
    wbufs = 3 if C <= 1150 else 2

    with TileContext(nc) as tc:
        with (
            tc.tile_pool(name="consts", bufs=1) as consts,
            tc.tile_pool(name="xp", bufs=1) as xp,
            tc.tile_pool(name="yaccp", bufs=1) as yaccp,
            tc.tile_pool(name="w1p", bufs=wbufs) as w1p,
            tc.tile_pool(name="w2p", bufs=wbufs) as w2p,
            tc.tile_pool(name="hp", bufs=wbufs) as hp,
            tc.tile_pool(name="psh", bufs=3, space="PSUM") as psh,
            tc.tile_pool(name="psy", bufs=3, space="PSUM") as psy,
        ):
            b1_sb = consts.tile([P, KF], f32, tag="b1")
            nc.sync.dma_start(b1_sb[:], b1v[:, :])
            b2_sb = consts.tile([P, KH], f32, tag="b2")
            nc.sync.dma_start(b2_sb[:], b2v[:, :])

            # Issue order tuned for the startup critical path: the first
            # matmul group needs xT[:, :, r0] and w1[fb=0, fc=0] only
            # (~1.9MB), so those DMAs are emitted first.
            xT_sb = xp.tile([P, KH, C], f32r, tag="xT")
            r0_, rw_ = tiles[0]
            for k in range(KH):
                nc.sync.dma_start(xT_sb[:, k, r0_:r0_ + rw_],
                                  xT[k * P:(k + 1) * P, r0_:r0_ + rw_])

            def load_w1(fb, fine):
                w1blk = w1p.tile([P, KH, FB], f32r, tag="w1blk")
                if fine:
                    # fc-major so the fc=0 columns (first psum group) land first
                    for fc in range(FC):
                        for k in range(KH):
                            nc.sync.dma_start(
                                w1blk[:, k, fc * P:(fc + 1) * P],
                                w1[k * P:(k + 1) * P,
                                   fb * FB + fc * P:fb * FB + (fc + 1) * P])
                else:
                    for k in range(KH):
                        nc.sync.dma_start(
                            w1blk[:, k, :],
                            w1[k * P:(k + 1) * P, fb * FB:(fb + 1) * FB])
                return w1blk

            def load_w2(fb):
                w2blk = w2p.tile([P, FC, H], f32r, tag="w2blk")
                for fc in range(FC):
                    nc.sync.dma_start(
                        w2blk[:, fc, :],
                        w2[fb * FB + fc * P:fb * FB + (fc + 1) * P, :])
                return w2blk

            first_blks = (load_w1(0, fine=True), load_w2(0))
            for (r0, rw) in tiles[1:]:
                for k in range(KH):
                    nc.sync.dma_start(xT_sb[:, k, r0:r0 + rw],
                                      xT[k * P:(k + 1) * P, r0:r0 + rw])

            yacc = yaccp.tile([P, KH, C], f32, tag="yacc")

            def body(first_blks=None, last=True):
                for fb in range(NFB):
                    if fb == 0 and first_blks is not None:
                        w1blk, w2blk = first_blks
                    else:
                        w1blk = load_w1(fb, fine=False)
                        w2blk = load_w2(fb)
                    for (r0, rw) in tiles:
                        h_sb = hp.tile([P, FC, max(t[1] for t in tiles)],
                                       f32r, tag="h")
                        # layer 1: hT[fc] = relu(W1blk.T @ xT + b1)
                        for fc in range(FC):
                            ph = psh.tile([P, rw], f32, tag="ph")
                            for k in range(KH):
                                nc.tensor.matmul(
                                    ph[:],
                                    w1blk[:, k, fc * P:(fc + 1) * P],
                                    xT_sb[:, k, r0:r0 + rw],
                                    start=(k == 0), stop=(k == KH - 1))
                            nc.scalar.activation(
                                h_sb[:, fc, :rw], ph[:],
                                mybir.ActivationFunctionType.Relu,
                                bias=b1_sb[:, fb * FC + fc:fb * FC + fc + 1])
                        # layer 2 partial: yacc[m] (+)= W2blk.T @ hT
                        for m in range(KH):
                            py = psy.tile([P, rw], f32, tag="py")
                            for fc in range(FC):
                                nc.tensor.matmul(
                                    py[:],
                                    w2blk[:, fc, m * P:(m + 1) * P],
                                    h_sb[:, fc, :rw],
                                    start=(fc == 0), stop=(fc == FC - 1))
                            if fb == 0:
                                # fold the layer-2 bias into the first partial
                                nc.scalar.activation(
                                    yacc[:, m, r0:r0 + rw], py[:],
                                    mybir.ActivationFunctionType.Identity,
                                    bias=b2_sb[:, m:m + 1])
                            else:
                                nc.vector.tensor_add(
                                    out=yacc[:, m, r0:r0 + rw],
                                    in0=yacc[:, m, r0:r0 + rw], in1=py[:])
                            if fb == NFB - 1 and last:
                                # writeback overlaps the remaining compute
                                nc.sync.dma_start(
                                    yT[m * P:(m + 1) * P, r0:r0 + rw],
                                    yacc[:, m, r0:r0 + rw])

            for i in range(reps - 1):
                body(first_blks if i == 0 else None, last=False)
            body(first_blks if reps == 1 else None, last=True)
    nc.finalize()
    return nc


# SBUF residency (xT + yacc at 64*C bytes/partition) caps per-launch capacity.
MAX_C = 1536


def _prepare(x, note_type_pos, W1, b1, W2, b2, cap):
    """Host-side routing: sort rows by expert, pad to capacity C (<= cap)."""
    ntp = np.asarray(note_type_pos).astype(np.int64)
    x = np.ascontiguousarray(np.asarray(x, dtype=np.float32))
    counts = np.bincount(ntp, minlength=N_EXPERTS)
    C = min(int(counts.max()), cap)
    n = -(-C // 512)
    rw = ((-(-C // n) + 15) // 16) * 16
    C = rw * n

    order = np.argsort(ntp, kind="stable")
    weights = []
    for e in range(N_EXPERTS):
        weights.append({
            "w1": np.ascontiguousarray(np.asarray(W1[e], dtype=np.float32)),
            "b1v": np.ascontiguousarray(
                np.asarray(b1[e], dtype=np.float32).reshape(KF, P).T),
            "w2": np.ascontiguousarray(np.asarray(W2[e], dtype=np.float32)),
            "b2v": np.ascontiguousarray(
                np.asarray(b2[e], dtype=np.float32).reshape(KH, P).T),
        })
    # chunk each expert's rows into groups of <= C; one SPMD launch per group
    launches = []
    off = 0
    expert_rows = []
    for e in range(N_EXPERTS):
        expert_rows.append(order[off:off + counts[e]])
        off += counts[e]
    n_launch = max(1, -(-int(counts.max()) // C))
    for g in range(n_launch):
        in_maps, row_idx = [], []
        for e in range(N_EXPERTS):
            rows = expert_rows[e][g * C:(g + 1) * C]
            row_idx.append(rows)
            xe = np.zeros((C, H), dtype=np.float32)
            if len(rows):
                xe[:len(rows)] = x[rows]
            in_maps.append({"xT": np.ascontiguousarray(xe.T), **weights[e]})
        launches.append((in_maps, row_idx))
    return launches, C


def kernel(x, note_type_pos, W1, b1, W2, b2):
    launches, C = _prepare(x, note_type_pos, W1, b1, W2, b2, cap=MAX_C)
    nc = build_expert_kernel(C)
    from concourse.bass_utils import run_bass_kernel_spmd
    T = np.asarray(x).shape[0]
    out = np.zeros((T, H), dtype=np.float32)
    for in_maps, row_idx in launches:
        res = run_bass_kernel_spmd(nc, in_maps, core_ids=list(range(N_EXPERTS)))
        for e in range(N_EXPERTS):
            rows = row_idx[e]
            if len(rows):
                out[rows] = res.results[e]["yT"].T[:len(rows)]
    return out
